# revision 8
# baseline (speedup 1.0000x reference)
"""Trainium2 Bass kernel for nn_NodeEncoder (GAT(1->256) + SAGE(256->128) + SAGE(128->128)).

Math factorization (exact): IN=1 makes the GAT layer rank-1 (g[n] * W1row), so
relu(GAT) is rank-2 and SAGE1 reduces to 5 per-node coefficients
C5=(P,Q,p,q,1); h2 = relu(C5 @ B5). Only SAGE2 needs 128-wide per-edge data.

Distribution: nodes sharded by contiguous ranges across 8 cores, degree-sorted
within each core (rank rho -> window w=rho//128, partition p=rho%128) so
per-window slot padding is small.

Phases:
  A: per-node g via host-replicated 2-hop x values ([x[n], x[N_in(n)]] padded
     to max-degree width) -- pure DVE/Scalar, no gathers.
  B: per-edge g[src] for every in-edge via the same 2-hop replication keyed by
     the edge's SOURCE, in destination-aligned slot layout; Sp/Sq = windowed
     row reduces. No gathers (SWDGE descriptor generation costs ~8ns/row on
     the Pool engine, which made gather-based variants 2-4ms).
  h2: each core builds h2 for its own nodes only (PE matmuls from C5) and an
     AllGather replicates the [N,128] f16 table.
  C: per-edge h2[src] rows fetched with batched dma_gather (128-edge tiles,
     4 source-quarter subsets so rows fit int16 indices); segment sums via
     one-hot matmuls accumulated in PSUM per destination window; final SAGE2
     weights applied per window. This phase's ~230k gather rows * 8ns is the
     dominant remaining cost.
"""

import os
import sys

if "/opt/trn_rl_repo" not in sys.path:
    sys.path.insert(0, "/opt/trn_rl_repo")

import numpy as np

import concourse.bacc as bacc
import concourse.bass as bass
import concourse.mybir as mybir
import concourse.tile as tile
from concourse.bass_utils import run_bass_kernel_spmd

NC = 8
NEG = 0.2          # leaky-relu slope (PyG GATConv default)
P = 128
F32 = mybir.dt.float32
F16 = mybir.dt.float16
I16 = mybir.dt.int16
Alu = mybir.AluOpType
Act = mybir.ActivationFunctionType

CB = 96            # phase-B gather chunk, slot columns (12288 idxs/call)
TMAX = 160        # phase-C tiles per window group (SBUF: 2 x 41KB/part)
CALL_TILES = 48    # small calls: ~5 fit the 1024-desc ring, overlapping desc-gen with drain

LAST_EXEC_NS = None


def _wrap_idx(flat):
    """Pack an int16 idx list into the SWDGE wrapped layout [128, n//16]:
    idx k lives at (partition k%16, col k//16), replicated across the eight
    16-partition groups."""
    n = flat.shape[0]
    assert n % 16 == 0
    w = flat.reshape(n // 16, 16).T.astype(np.int16)   # [16, n//16]
    return np.tile(w, (8, 1))


def _host_prep(x, edge_index):
    N = x.shape[0]
    E = edge_index.shape[1]
    Nl = N // NC
    GC = -(-Nl // P)
    Nlp = P * GC
    NT = NC * Nlp
    QS = NT // 4

    src = np.ascontiguousarray(edge_index[0]).astype(np.int64)
    dst = np.ascontiguousarray(edge_index[1]).astype(np.int64)
    deg = np.bincount(dst, minlength=N).astype(np.int64)
    xf = np.asarray(x[:, 0], np.float32)
    D1 = int(deg.max()) + 1          # self + in-neighbours, fixed width

    # degree-sorted local ranks
    n_all = np.arange(N)
    c_all = n_all // Nl
    rho = np.empty(N, np.int64)
    for c in range(NC):
        ids = np.arange(c * Nl, (c + 1) * Nl)
        order = np.argsort(-deg[ids], kind="stable")
        rho[ids[order]] = np.arange(Nl)
    gid = c_all * Nlp + rho
    w_all = rho // P
    p_all = rho % P

    # grids [NC, P, GC]
    x_grid = np.zeros((NC, P, GC), np.float32)
    x_grid[c_all, p_all, w_all] = xf
    deg_grid = np.zeros((NC, P, GC), np.int64)
    deg_grid[c_all, p_all, w_all] = deg

    # phase A/B slot geometry: Kb[w] = max degree in window (over cores)
    Kb = deg_grid.max(axis=1).max(axis=0)              # [GC]
    SKB = int(Kb.sum())
    SKBp = SKB
    baseB = np.zeros(GC + 1, np.int64)
    np.cumsum(Kb, out=baseB[1:])

    # per-edge rank within destination
    okey = np.argsort(gid[dst], kind="stable")
    sd = gid[dst][okey]
    erank_o = np.arange(E) - np.searchsorted(sd, sd)
    erank = np.empty(E, np.int64)
    erank[okey] = erank_o

    dcore = dst // Nl
    dw = rho[dst] // P
    dp = rho[dst] % P
    slot_col = baseB[dw] + erank                       # [E]

    # 2-hop neighbourhood table: row n = [x[n], x of in-neighbours, 0 pad]
    nbr_x = np.zeros((N, D1), np.float16)
    nbr_x[:, 0] = xf.astype(np.float16)
    nbr_x[dst, 1 + erank] = xf[src].astype(np.float16)

    # phase C tile geometry
    q_src = gid[src] // QS                              # [E] 0..3
    ewq = np.zeros((NC, GC, 4), np.int64)
    np.add.at(ewq, (dcore, dw, q_src), 1)
    Kwq = -(-ewq.max(axis=0) // P)                      # [GC, 4]

    # window groups (greedy by total tiles)
    wtiles = Kwq.sum(axis=1)                            # [GC]
    groups = []
    w0 = 0
    while w0 < GC:
        w1 = w0 + 1
        tot = int(wtiles[w0])
        while w1 < GC and tot + int(wtiles[w1]) <= TMAX:
            tot += int(wtiles[w1])
            w1 += 1
        groups.append((w0, w1))
        w0 = w1

    # tile column order: group -> quarter -> window
    col_of = np.zeros((GC, 4), np.int64)
    calls = []          # (grp_idx, q, colbase, ntiles) per dma_gather call
    grp_info = []       # per group: (w0, w1, colbase, ntiles)
    off = 0
    for gi, (w0, w1) in enumerate(groups):
        gbase = off
        for q in range(4):
            qbase = off
            for w in range(w0, w1):
                col_of[w, q] = off
                off += int(Kwq[w, q])
            ntq = off - qbase
            t0 = qbase
            while ntq > 0:
                n = min(ntq, CALL_TILES)
                calls.append((gi, q, t0, n))
                t0 += n
                ntq -= n
        grp_info.append((w0, w1, gbase, off - gbase))
    T = off

    # per-edge tile slots: rank within (core, window, quarter)
    okey2 = np.argsort((gid[dst] * 4 + q_src), kind="stable")
    sk2 = (gid[dst] * 4 + q_src)[okey2]
    # rank within (dst, quarter) -- but we need rank within (core, window,
    # quarter) across all dsts of the window.  Sort by (core, w, q, anything):
    key3 = (dcore * GC + dw) * 4 + q_src
    okey3 = np.argsort(key3, kind="stable")
    sk3 = key3[okey3]
    rank3_o = np.arange(E) - np.searchsorted(sk3, sk3)
    rank3 = np.empty(E, np.int64)
    rank3[okey3] = rank3_o
    tile_of = col_of[dw, q_src] + rank3 // P
    tslot = rank3 % P

    meta = []
    for c in range(NC):
        em = dcore == c
        sc, dc_ = src[em], dst[em]
        spc, col = dp[em], slot_col[em]

        # node-grid 2-hop arrays (phase A: per-node g)
        ids = np.arange(c * Nl, (c + 1) * Nl)
        x2hN = np.zeros((P, GC, D1), np.float16)
        dcntN = np.ones((P, GC), np.float16)
        x2hN[p_all[ids], w_all[ids]] = nbr_x[ids]
        dcntN[p_all[ids], w_all[ids]] = (deg[ids] + 1).astype(np.float16)

        # edge-slot 2-hop arrays (phase B: per-edge g[src])
        x2h = np.zeros((P, SKBp, D1), np.float16)
        dcnt = np.ones((P, SKBp), np.float16)
        x2h[spc, col] = nbr_x[sc]
        dcnt[spc, col] = (deg[sc] + 1).astype(np.float16)

        cti, csl = tile_of[em], tslot[em]
        cidx = np.zeros((P, T), np.int64)
        cdlo = np.full((P, T), 200.0, np.float16)
        cdinv = np.zeros((P, T), np.float16)
        cidx[csl, cti] = gid[sc] - q_src[em] * QS
        cdlo[csl, cti] = dp[em].astype(np.float16)
        cdinv[csl, cti] = (1.0 / np.maximum(deg[dc_], 1)).astype(np.float16)
        cidx_p = _wrap_idx(cidx.T.reshape(-1))          # [128, T*8]

        meta.append(dict(x2hN=x2hN.reshape(P, GC * D1),
                         dcntN=dcntN,
                         x2h=x2h.reshape(P, SKBp * D1),
                         dcnt=dcnt,
                         cidx=cidx_p, cdlo=cdlo, cdinv=cdinv,
                         deg_inv=np.where(deg_grid[c] > 0,
                                          1.0 / np.maximum(deg_grid[c], 1),
                                          1.0).astype(np.float32)))

    # per-window tile runs for the builder
    wruns = []
    for w in range(GC):
        runs = [(int(col_of[w, q]), int(Kwq[w, q]))
                for q in range(4) if Kwq[w, q] > 0]
        wruns.append(runs)

    # phase-B window chunks (bounded slot columns per chunk)
    CCOL = 128
    bchunks = []
    w0 = 0
    while w0 < GC:
        w1 = w0 + 1
        tot = int(Kb[w0])
        while w1 < GC and tot + int(Kb[w1]) <= CCOL:
            tot += int(Kb[w1])
            w1 += 1
        bchunks.append((w0, w1, int(baseB[w0]), tot))
        w0 = w1

    layout = dict(N=N, Nl=Nl, GC=GC, Nlp=Nlp, NT=NT, QS=QS, D1=D1,
                  SKB=SKB, SKBp=SKBp, Kb=Kb, baseB=baseB, bchunks=bchunks,
                  T=T, calls=calls, grp_info=grp_info, wruns=wruns,
                  gid=gid)
    return meta, layout


def _build_program(layout, H1, H2, OUT):
    GC, Nlp, NT, SKBp = layout["GC"], layout["Nlp"], layout["NT"], layout["SKBp"]
    SKB, Kb, baseB = layout["SKB"], layout["Kb"], layout["baseB"]
    T, calls, grp_info, wruns = (layout["T"], layout["calls"],
                                 layout["grp_info"], layout["wruns"])
    D1, bchunks = layout["D1"], layout["bchunks"]
    CCOL = max(cc for (_, _, _, cc) in bchunks)
    KH = H1 // P
    maxkw = max((sum(k for _, k in runs) for runs in wruns), default=1) or 1

    nc = bacc.Bacc("TRN2", target_bir_lowering=False, debug=False,
                   num_devices=NC, num_swdge_queues=4)

    def din(name, shape, dt):
        return nc.dram_tensor(name, shape, dt, kind="ExternalInput").ap()

    x2hN_t = din("x2hN", [P, GC * D1], F16)
    dcntN_t = din("dcntN", [P, GC], F16)
    x2h_t = din("x2h", [P, SKBp * D1], F16)
    dcnt_t = din("dcnt", [P, SKBp], F16)
    cidx_t = din("cidx", [P, T * 8], I16)
    cdlo_t = din("cdlo", [P, T], F16)
    cdinv_t = din("cdinv", [P, T], F16)
    deg_inv_t = din("deg_inv", [P, GC], F32)
    W1_t = din("W1", [1, H1], F32)
    att_s_t = din("att_src", [H1], F32)
    att_d_t = din("att_dst", [H1], F32)
    Wl1_t = din("Wl1", [H1, H2], F32)
    bl1_t = din("bl1", [H2], F32)
    Wr1_t = din("Wr1", [H1, H2], F32)
    Wl2_t = din("Wl2", [H2, OUT], F32)
    bl2_t = din("bl2", [OUT], F32)
    Wr2_t = din("Wr2", [H2, OUT], F32)
    out_t = nc.dram_tensor("out", [P, Nlp], F32, kind="ExternalOutput").ap()

    with tile.TileContext(nc) as tc:
        with (
            tc.tile_pool(name="dram", bufs=1, space="DRAM") as dram,
            tc.tile_pool(name="const", bufs=1) as constp,
            tc.tile_pool(name="grids", bufs=1) as gridp,
        ):
            # ---------------- phase 0: scalars and weight products ----------
            ph0 = tc.tile_pool(name="psum_s", bufs=2, space="PSUM")
            psum_s = ph0.__enter__()
            w_col = constp.tile([P, KH], F32)
            nc.sync.dma_start(w_col[:], W1_t.rearrange("o (j p) -> p (o j)", p=P))
            att_s = constp.tile([P, KH], F32)
            nc.sync.dma_start(att_s[:], att_s_t.rearrange("(j p) -> p j", p=P))
            att_d = constp.tile([P, KH], F32)
            nc.sync.dma_start(att_d[:], att_d_t.rearrange("(j p) -> p j", p=P))

            m23 = constp.tile([P, 2 * KH], F32)
            nc.vector.tensor_mul(out=m23[:, 0:KH], in0=w_col[:], in1=att_s[:])
            nc.vector.tensor_mul(out=m23[:, KH:2 * KH], in0=w_col[:], in1=att_d[:])
            ones_col = constp.tile([P, 1], F32)
            nc.vector.memset(ones_col[:], 1.0)
            csd_ps = psum_s.tile([1, 2 * KH], F32, space="PSUM")
            nc.tensor.matmul(csd_ps[:], lhsT=ones_col[:], rhs=m23[:], start=True, stop=True)
            csd4 = constp.tile([1, 2 * KH], F32)
            nc.vector.tensor_copy(out=csd4[:], in_=csd_ps[:])
            csd2 = constp.tile([1, 2], F32)
            nc.vector.tensor_reduce(
                out=csd2[:], in_=csd4[:].rearrange("o (a j) -> o a j", a=2),
                axis=mybir.AxisListType.X, op=Alu.add)
            ones_row = constp.tile([1, P], F32)
            nc.vector.memset(ones_row[:], 1.0)
            csd_bps = psum_s.tile([P, 2], F32, space="PSUM")
            nc.tensor.matmul(csd_bps[:], lhsT=ones_row[:], rhs=csd2[:], start=True, stop=True)
            csd_col = constp.tile([P, 2], F32)
            nc.vector.tensor_copy(out=csd_col[:], in_=csd_bps[:])
            cs_col = csd_col[:, 0:1]
            cd_col = csd_col[:, 1:2]
            cscd_col = constp.tile([P, 1], F32)
            nc.vector.tensor_add(out=cscd_col[:], in0=cs_col, in1=cd_col)

            # u/v columns and B5 = [u@Wl1; v@Wl1; u@Wr1; v@Wr1; bl1]
            uv = constp.tile([P, 2 * KH], F32)
            uvv = uv[:].rearrange("p (j two) -> p j two", two=2)
            nc.vector.tensor_scalar_max(out=uvv[:, :, 0], in0=w_col[:], scalar1=0.0)
            nc.vector.tensor_scalar(out=uvv[:, :, 1], in0=w_col[:], scalar1=-1.0,
                                    scalar2=0.0, op0=Alu.mult, op1=Alu.max)
            b5_dram = dram.tile([5, H2], F32)
            wlr = constp.tile([P, 2 * H2], F32, tag="wlr")
            abcd_ps = psum_s.tile([2, 2 * H2], F32, space="PSUM", tag="ab")
            for j in range(KH):
                nc.sync.dma_start(wlr[:, 0:H2], Wl1_t[j * P:(j + 1) * P, :])
                nc.sync.dma_start(wlr[:, H2:2 * H2], Wr1_t[j * P:(j + 1) * P, :])
                nc.tensor.matmul(abcd_ps[:], lhsT=uv[:, 2 * j:2 * j + 2], rhs=wlr[:],
                                 start=(j == 0), stop=(j == KH - 1))
            abcd_sb = constp.tile([2, 2 * H2], F32)
            nc.vector.tensor_copy(out=abcd_sb[:], in_=abcd_ps[:])
            nc.sync.dma_start(
                b5_dram[0:4, :].rearrange("(s r) f -> r s f", s=2),
                abcd_sb[:].rearrange("r (s f) -> r s f", s=2))
            nc.sync.dma_start(b5_dram[4:5, :], bl1_t.rearrange("(o f) -> o f", o=1))
            B5 = constp.tile([5, H2], F32)
            nc.sync.dma_start(B5[:], b5_dram[:])

            Wl2_h = constp.tile([H2, OUT], F16)
            wl2_f = constp.tile([H2, OUT], F32, tag="wtmp")
            nc.sync.dma_start(wl2_f[:], Wl2_t[:])
            nc.vector.tensor_copy(out=Wl2_h[:], in_=wl2_f[:])
            Wr2_h = constp.tile([H2, OUT], F16)
            wr2_f = constp.tile([H2, OUT], F32, tag="wtmp")
            nc.sync.dma_start(wr2_f[:], Wr2_t[:])
            nc.vector.tensor_copy(out=Wr2_h[:], in_=wr2_f[:])
            bl2_col = constp.tile([P, 1], F32)
            nc.sync.dma_start(bl2_col[:], bl2_t.rearrange("(p o) -> p o", o=1))

            iotaD_i = constp.tile([P, D1], mybir.dt.int32)
            nc.gpsimd.iota(iotaD_i[:], pattern=[[1, D1]], base=0, channel_multiplier=0)
            iotaD = constp.tile([P, D1], F16)
            nc.vector.tensor_copy(out=iotaD[:], in_=iotaD_i[:])
            iota128_i = constp.tile([P, P], mybir.dt.int32)
            nc.gpsimd.iota(iota128_i[:], pattern=[[1, P]], base=0, channel_multiplier=0)
            iota128h = constp.tile([P, P], F16)
            nc.vector.tensor_copy(out=iota128h[:], in_=iota128_i[:])
            identity = constp.tile([P, P], F32)
            from concourse.masks import make_identity
            make_identity(nc, identity[:])
            ph0.__exit__(None, None, None)

            # ---------------- persistent grids ----------------
            deg_inv = gridp.tile([P, GC], F32)
            nc.sync.dma_start(deg_inv[:], deg_inv_t[:])
            h2T = gridp.tile([P, Nlp], F16)

            h2_loc = dram.tile([Nlp, H2], F16)
            h2_tab = dram.tile([NC, Nlp, H2], F16, addr_space="Shared")
            c5_loc = dram.tile([5, Nlp], F32)

            # ---------------- phase A: per-node g via 2-hop slots ----------
            g_grid = gridp.tile([P, GC], F32)
            with tc.tile_pool(name="ph_a", bufs=1) as pa:
                x2n = pa.tile([P, GC * D1], F16)
                nc.sync.dma_start(x2n[:], x2hN_t[:])
                dcn = pa.tile([P, GC], F16)
                nc.sync.dma_start(dcn[:], dcntN_t[:])
                zN = pa.tile([P, GC * D1], F16)
                nc.vector.tensor_scalar(out=zN[:], in0=x2n[:], scalar1=cs_col,
                                        scalar2=None, op0=Alu.mult)
                x0v = (x2n[:].rearrange("p (c j) -> p c j", j=D1)[:, :, 0:1]
                       .to_broadcast([P, GC, D1]))
                zN3 = zN[:].rearrange("p (c j) -> p c j", j=D1)
                nc.vector.scalar_tensor_tensor(out=zN3, in0=x0v, scalar=cd_col,
                                               in1=zN3, op0=Alu.mult, op1=Alu.add)
                nc.vector.scalar_tensor_tensor(out=zN[:], in0=zN[:], scalar=NEG,
                                               in1=zN[:], op0=Alu.mult, op1=Alu.max)
                eeN = pa.tile([P, GC * D1], F32)
                nc.scalar.activation(eeN[:], zN[:], Act.Exp)
                mkN = pa.tile([P, GC * D1], F16)
                mkN3 = mkN[:].rearrange("p (c j) -> p c j", j=D1)
                nc.vector.tensor_tensor(
                    out=mkN3,
                    in0=iotaD[:].unsqueeze(1).to_broadcast([P, GC, D1]),
                    in1=dcn[:].unsqueeze(2).to_broadcast([P, GC, D1]),
                    op=Alu.is_lt)
                nc.vector.tensor_mul(out=eeN[:], in0=eeN[:], in1=mkN[:])
                SN = pa.tile([P, GC], F32)
                nc.vector.tensor_reduce(
                    out=SN[:], in_=eeN[:].rearrange("p (c j) -> p c j", j=D1),
                    axis=mybir.AxisListType.X, op=Alu.add)
                nc.vector.tensor_mul(out=eeN[:], in0=eeN[:], in1=x2n[:])
                WN = pa.tile([P, GC], F32)
                nc.vector.tensor_reduce(
                    out=WN[:], in_=eeN[:].rearrange("p (c j) -> p c j", j=D1),
                    axis=mybir.AxisListType.X, op=Alu.add)
                nc.vector.reciprocal(out=g_grid[:], in_=SN[:])
                nc.vector.tensor_mul(out=g_grid[:], in0=g_grid[:], in1=WN[:])

            # ---------------- phase B: per-edge g[src] via 2-hop slots ------
            Sp_grid = gridp.tile([P, GC], F32)
            Sq_grid = gridp.tile([P, GC], F32)
            with tc.tile_pool(name="ph_b", bufs=2) as pb, \
                 tc.tile_pool(name="ph_b1", bufs=1) as pb1, \
                 tc.tile_pool(name="psum_b", bufs=2, space="PSUM") as psum_b:
                pe_grid = pb1.tile([P, SKBp], F16)
                qe_grid = pb1.tile([P, SKBp], F16)
                for (w0b, w1b, c0, cc) in bchunks:
                    x2c = pb.tile([P, CCOL * D1], F16, tag="x2c")
                    nc.sync.dma_start(x2c[:, :cc * D1],
                                      x2h_t[:, c0 * D1:(c0 + cc) * D1])
                    dcc = pb.tile([P, CCOL], F16, tag="dcc")
                    nc.sync.dma_start(dcc[:, :cc], dcnt_t[:, c0:c0 + cc])
                    x2c3 = x2c[:, :cc * D1].rearrange("p (c j) -> p c j", j=D1)
                    zc = pb.tile([P, CCOL * D1], F16, tag="zc")
                    zc3 = zc[:, :cc * D1].rearrange("p (c j) -> p c j", j=D1)
                    nc.vector.tensor_scalar(out=zc[:, :cc * D1],
                                            in0=x2c[:, :cc * D1],
                                            scalar1=cs_col, scalar2=None,
                                            op0=Alu.mult)
                    x0c = x2c3[:, :, 0:1].to_broadcast([P, cc, D1])
                    nc.vector.scalar_tensor_tensor(out=zc3, in0=x0c,
                                                   scalar=cd_col, in1=zc3,
                                                   op0=Alu.mult, op1=Alu.add)
                    nc.vector.scalar_tensor_tensor(out=zc[:, :cc * D1],
                                                   in0=zc[:, :cc * D1],
                                                   scalar=NEG,
                                                   in1=zc[:, :cc * D1],
                                                   op0=Alu.mult, op1=Alu.max)
                    eec = pb.tile([P, CCOL * D1], F32, tag="eec")
                    nc.scalar.activation(eec[:, :cc * D1], zc[:, :cc * D1],
                                         Act.Exp)
                    zc3 = zc[:, :cc * D1].rearrange("p (c j) -> p c j", j=D1)
                    nc.vector.tensor_tensor(
                        out=zc3,
                        in0=iotaD[:].unsqueeze(1).to_broadcast([P, cc, D1]),
                        in1=dcc[:, :cc].unsqueeze(2).to_broadcast([P, cc, D1]),
                        op=Alu.is_lt)
                    nc.vector.tensor_mul(out=eec[:, :cc * D1],
                                         in0=eec[:, :cc * D1],
                                         in1=zc[:, :cc * D1])
                    Sc = pb.tile([P, CCOL], F32, tag="Sc")
                    eec3 = eec[:, :cc * D1].rearrange("p (c j) -> p c j", j=D1)
                    nc.vector.tensor_reduce(out=Sc[:, :cc], in_=eec3,
                                            axis=mybir.AxisListType.X,
                                            op=Alu.add)
                    nc.vector.tensor_mul(out=eec[:, :cc * D1],
                                         in0=eec[:, :cc * D1],
                                         in1=x2c[:, :cc * D1])
                    Wc = pb.tile([P, CCOL], F32, tag="Wc")
                    nc.vector.tensor_reduce(out=Wc[:, :cc], in_=eec3,
                                            axis=mybir.AxisListType.X,
                                            op=Alu.add)
                    gec = pb.tile([P, CCOL], F32, tag="gec")
                    nc.vector.reciprocal(out=gec[:, :cc], in_=Sc[:, :cc])
                    nc.vector.tensor_mul(out=gec[:, :cc], in0=gec[:, :cc],
                                         in1=Wc[:, :cc])
                    nc.vector.tensor_scalar_max(out=pe_grid[:, c0:c0 + cc],
                                                in0=gec[:, :cc], scalar1=0.0)
                    nc.vector.tensor_scalar(out=qe_grid[:, c0:c0 + cc],
                                            in0=gec[:, :cc], scalar1=-1.0,
                                            scalar2=0.0, op0=Alu.mult,
                                            op1=Alu.max)
                for w in range(GC):
                    kb = int(Kb[w])
                    o0 = int(baseB[w])
                    if kb == 0:
                        nc.vector.memset(Sp_grid[:, w:w + 1], 0.0)
                        nc.vector.memset(Sq_grid[:, w:w + 1], 0.0)
                        continue
                    nc.vector.tensor_reduce(
                        out=Sp_grid[:, w:w + 1],
                        in_=pe_grid[:, o0:o0 + kb].rearrange("p (o k) -> p o k", o=1),
                        axis=mybir.AxisListType.X, op=Alu.add)
                    nc.vector.tensor_reduce(
                        out=Sq_grid[:, w:w + 1],
                        in_=qe_grid[:, o0:o0 + kb].rearrange("p (o k) -> p o k", o=1),
                        axis=mybir.AxisListType.X, op=Alu.add)

                # coefficient grids -> transposed -> c5_loc rows
                cP = pb1.tile([P, GC], F32)
                nc.vector.tensor_mul(out=cP[:], in0=Sp_grid[:], in1=deg_inv[:])
                cQ = pb1.tile([P, GC], F32)
                nc.vector.tensor_mul(out=cQ[:], in0=Sq_grid[:], in1=deg_inv[:])
                cp = pb1.tile([P, GC], F32)
                nc.vector.tensor_scalar_max(out=cp[:], in0=g_grid[:], scalar1=0.0)
                cq = pb1.tile([P, GC], F32)
                nc.vector.tensor_scalar(out=cq[:], in0=g_grid[:], scalar1=-1.0,
                                        scalar2=0.0, op0=Alu.mult, op1=Alu.max)
                for j, grid in enumerate((cP, cQ, cp, cq)):
                    tp = psum_b.tile([GC, P], F32, space="PSUM", tag="tp")
                    nc.tensor.matmul(tp[:], lhsT=grid[:], rhs=identity[:],
                                     start=True, stop=True)
                    tps = pb.tile([GC, P], F32, tag="tps")
                    nc.vector.tensor_copy(out=tps[:], in_=tp[:])
                    nc.sync.dma_start(
                        c5_loc[j:j + 1, :].rearrange("o (w e) -> (o w) e", e=P),
                        tps[:])
                ones_t = pb1.tile([GC, P], F32)
                nc.vector.memset(ones_t[:], 1.0)
                nc.sync.dma_start(
                    c5_loc[4:5, :].rearrange("o (w e) -> (o w) e", e=P),
                    ones_t[:])

            # ---------------- h2 build (local nodes only) ----------------
            with tc.tile_pool(name="h2p", bufs=3) as h2p, \
                 tc.tile_pool(name="h2c", bufs=1) as h2c, \
                 tc.tile_pool(name="psum_h", bufs=3, space="PSUM") as psum_h:
                c5_sb = h2c.tile([5, Nlp], F32)
                nc.sync.dma_start(c5_sb[:], c5_loc[:])
                GB = 4
                for w0 in range(0, GC, GB):
                    nw = min(GB, GC - w0)
                    hp = psum_h.tile([P, GB * H2], F32, space="PSUM", tag="hp")
                    for j in range(nw):
                        w = w0 + j
                        nc.tensor.matmul(
                            hp[:, j * H2:(j + 1) * H2],
                            lhsT=c5_sb[:, w * P:(w + 1) * P],
                            rhs=B5[:], start=True, stop=True)
                    ht = h2p.tile([P, GB * H2], F16, tag="ht")
                    nc.vector.tensor_scalar_max(out=ht[:, :nw * H2],
                                                in0=hp[:, :nw * H2], scalar1=0.0)
                    nc.sync.dma_start(
                        h2_loc[w0 * P:(w0 + nw) * P, :]
                            .rearrange("(j p) f -> p j f", p=P),
                        ht[:, :nw * H2].rearrange("p (j f) -> p j f", f=H2))
                # transposed local h2 for the Wr2 term
                for a in range(0, Nlp, 512):
                    wd = min(512, Nlp - a)
                    hp2 = psum_h.tile([P, 512], F32, space="PSUM", tag="hp2")
                    nc.tensor.matmul(hp2[:, :wd], lhsT=B5[:], rhs=c5_sb[:, a:a + wd],
                                     start=True, stop=True)
                    nc.vector.tensor_scalar_max(out=h2T[:, a:a + wd],
                                                in0=hp2[:, :wd], scalar1=0.0)

            nc.gpsimd.collective_compute(
                "AllGather", Alu.bypass,
                replica_groups=[list(range(NC))],
                ins=[h2_loc.opt()], outs=[h2_tab.opt()])

            # ---------------- phase C ----------------
            with tc.tile_pool(name="ph_c", bufs=2) as pc, \
                 tc.tile_pool(name="ph_cm", bufs=3) as pcm, \
                 tc.tile_pool(name="ph_c1", bufs=1) as pc1, \
                 tc.tile_pool(name="psum_c", bufs=4, space="PSUM") as psum_c:
                cdlo_sb = pc1.tile([P, T], F16)
                nc.sync.dma_start(cdlo_sb[:], cdlo_t[:])
                cdinv_sb = pc1.tile([P, T], F16)
                nc.sync.dma_start(cdinv_sb[:], cdinv_t[:])

                h2q = [h2_tab[2 * q:2 * q + 2].rearrange("a r e -> (a r) e")
                       for q in range(4)]
                calls_by_grp = {}
                for (gi, q, t0, ntl) in calls:
                    calls_by_grp.setdefault(gi, []).append((q, t0, ntl))

                _gq = [0]
                for gi, (w0, w1, gbase, gtiles) in enumerate(grp_info):
                    if gtiles == 0:
                        vt = None
                    else:
                        vt = pc.tile([P, TMAX * P], F16, tag="vt")
                        for (q, t0, ntl) in calls_by_grp.get(gi, []):
                            ci = pcm.tile([P, CALL_TILES * 8], I16, tag="ci")
                            nc.sync.dma_start(ci[:, :ntl * 8],
                                              cidx_t[:, t0 * 8:(t0 + ntl) * 8])
                            nc.gpsimd.dma_gather(
                                vt[:, (t0 - gbase) * P:(t0 - gbase + ntl) * P]
                                    .rearrange("p (t e) -> p t e", e=P),
                                h2q[q], ci[:, :ntl * 8],
                                num_idxs=ntl * P, num_idxs_reg=ntl * P,
                                elem_size=P, single_packet=False,
                                queue_num=_gq[0] % 4)
                            _gq[0] += 1
                        # scale by 1/deg (zeroes padding slots)
                        nc.vector.tensor_tensor(
                            out=vt[:, :gtiles * P].rearrange("p (t e) -> p t e", e=P),
                            in0=vt[:, :gtiles * P].rearrange("p (t e) -> p t e", e=P),
                            in1=cdinv_sb[:, gbase:gbase + gtiles].unsqueeze(2)
                                .to_broadcast([P, gtiles, P]),
                            op=Alu.mult)
                    for w in range(w0, w1):
                        runs = wruns[w]
                        ktot = sum(k for _, k in runs)
                        if ktot > 0:
                            mt = pcm.tile([P, maxkw * P], F16, tag="mt")
                            mo = 0
                            for (t0, k) in runs:
                                nc.vector.tensor_tensor(
                                    out=mt[:, mo * P:(mo + k) * P]
                                        .rearrange("p (t e) -> p t e", e=P),
                                    in0=cdlo_sb[:, t0:t0 + k].unsqueeze(2)
                                        .to_broadcast([P, k, P]),
                                    in1=iota128h[:].unsqueeze(1)
                                        .to_broadcast([P, k, P]),
                                    op=Alu.is_equal)
                                mo += k
                            yp = psum_c.tile([P, P], F32, space="PSUM", tag="yp")
                            mo = 0
                            ti = 0
                            for (t0, k) in runs:
                                for t in range(k):
                                    nc.tensor.matmul(
                                        yp[:],
                                        lhsT=vt[:, (t0 - gbase + t) * P:
                                                (t0 - gbase + t + 1) * P],
                                        rhs=mt[:, (mo + t) * P:(mo + t + 1) * P],
                                        start=(ti == 0),
                                        stop=(ti == ktot - 1))
                                    ti += 1
                                mo += k
                            ys = pcm.tile([P, P], F16, tag="ys")
                            nc.vector.tensor_copy(out=ys[:], in_=yp[:])
                        op = psum_c.tile([P, P], F32, space="PSUM", tag="op")
                        if ktot > 0:
                            nc.tensor.matmul(op[:], lhsT=Wl2_h[:], rhs=ys[:],
                                             start=True, stop=False)
                            nc.tensor.matmul(op[:], lhsT=Wr2_h[:],
                                             rhs=h2T[:, w * P:(w + 1) * P],
                                             start=False, stop=True)
                        else:
                            nc.tensor.matmul(op[:], lhsT=Wr2_h[:],
                                             rhs=h2T[:, w * P:(w + 1) * P],
                                             start=True, stop=True)
                        ow = pcm.tile([P, P], F32, tag="ow")
                        nc.scalar.activation(ow[:], op[:], Act.Identity,
                                             bias=bl2_col[:])
                        nc.sync.dma_start(out_t[:, w * P:(w + 1) * P], ow[:])

    nc.compile()
    return nc


def kernel(**inputs):
    x = np.asarray(inputs["x"], np.float32)
    edge_index = np.asarray(inputs["edge_index"])
    b1 = np.asarray(inputs["b1"], np.float32)
    assert float(np.abs(b1).max()) == 0.0, "kernel factorization requires b1 == 0"

    meta, layout = _host_prep(x, edge_index)
    H1 = inputs["W1"].shape[1]
    H2 = inputs["Wl1"].shape[1]
    OUT = inputs["Wl2"].shape[1]

    nc = _build_program(layout, H1, H2, OUT)

    shared = dict(
        W1=np.asarray(inputs["W1"], np.float32),
        att_src=np.asarray(inputs["att_src"], np.float32),
        att_dst=np.asarray(inputs["att_dst"], np.float32),
        Wl1=np.asarray(inputs["Wl1"], np.float32),
        bl1=np.asarray(inputs["bl1"], np.float32),
        Wr1=np.asarray(inputs["Wr1"], np.float32),
        Wl2=np.asarray(inputs["Wl2"], np.float32),
        bl2=np.asarray(inputs["bl2"], np.float32),
        Wr2=np.asarray(inputs["Wr2"], np.float32),
    )
    in_maps = []
    for c in range(NC):
        m = dict(shared)
        for k2 in ("x2hN", "dcntN", "x2h", "dcnt", "cidx", "cdlo",
                   "cdinv", "deg_inv"):
            m[k2] = meta[c][k2]
        in_maps.append(m)

    trace = bool(os.environ.get("KERNEL_TRACE"))
    if trace:
        try:
            import trn_agent_boot.trn_boot as _tb
            try:
                from antenv.axon_hooks import set_axon_ntff_profile_hook
            except ImportError:
                import types
                import antenv
                _m = types.ModuleType("antenv.axon_hooks")
                _h = {}
                _m.set_axon_ntff_profile_hook = lambda hk: _h.__setitem__("h", hk)
                _m.get_axon_ntff_profile_hook = lambda: _h.get("h")
                sys.modules["antenv.axon_hooks"] = _m
                antenv.axon_hooks = _m
                set_axon_ntff_profile_hook = _m.set_axon_ntff_profile_hook

            set_axon_ntff_profile_hook(
                _tb._ntff_profile_via_ctypes("/opt/axon/libaxon_pjrt.so"))
        except Exception:
            trace = False
    res = run_bass_kernel_spmd(nc, in_maps, core_ids=list(range(NC)), trace=trace)
    global LAST_EXEC_NS
    LAST_EXEC_NS = res.exec_time_ns

    N, Nlp, gid = layout["N"], layout["Nlp"], layout["gid"]
    full = np.concatenate([res.results[c]["out"].T for c in range(NC)], axis=0)
    return np.ascontiguousarray(full[gid]).astype(np.float32)


# revision 9
# speedup vs baseline: 1.1073x; 1.1073x over previous
"""Trainium2 Bass kernel for nn_NodeEncoder (GAT(1->256) + SAGE(256->128) + SAGE(128->128)).

Math factorization (exact): IN=1 makes the GAT layer rank-1 (g[n] * W1row), so
relu(GAT) is rank-2 and SAGE1 reduces to 5 per-node coefficients
C5=(P,Q,p,q,1); h2 = relu(C5 @ B5). Only SAGE2 needs 128-wide per-edge data.

Distribution: nodes sharded by contiguous ranges across 8 cores, degree-sorted
within each core (rank rho -> window w=rho//128, partition p=rho%128) so
per-window slot padding is small.

Phases:
  A: per-node g via host-replicated 2-hop x values ([x[n], x[N_in(n)]] padded
     to max-degree width) -- pure DVE/Scalar, no gathers.
  B: per-edge g[src] for every in-edge via the same 2-hop replication keyed by
     the edge's SOURCE, in destination-aligned slot layout; Sp/Sq = windowed
     row reduces. No gathers (SWDGE descriptor generation costs ~8ns/row on
     the Pool engine, which made gather-based variants 2-4ms).
  h2: each core builds h2 for its own nodes only (PE matmuls from C5) and an
     AllGather replicates the [N,128] f16 table.
  C: per-edge h2[src] rows fetched with batched dma_gather (128-edge tiles,
     4 source-quarter subsets so rows fit int16 indices); segment sums via
     one-hot matmuls accumulated in PSUM per destination window; final SAGE2
     weights applied per window. This phase's ~230k gather rows * 8ns is the
     dominant remaining cost.
"""

import os
import sys

if "/opt/trn_rl_repo" not in sys.path:
    sys.path.insert(0, "/opt/trn_rl_repo")

import numpy as np

import concourse.bacc as bacc
import concourse.bass as bass
import concourse.mybir as mybir
import concourse.tile as tile
from concourse.bass_utils import run_bass_kernel_spmd

NC = 8
NEG = 0.2          # leaky-relu slope (PyG GATConv default)
P = 128
F32 = mybir.dt.float32
F16 = mybir.dt.float16
I16 = mybir.dt.int16
Alu = mybir.AluOpType
Act = mybir.ActivationFunctionType

CB = 96            # phase-B gather chunk, slot columns (12288 idxs/call)
TMAX = 160        # phase-C tiles per window group (SBUF: 2 x 41KB/part)
CALL_TILES = 24    # small calls: ~5 fit the 1024-desc ring, overlapping desc-gen with drain

LAST_EXEC_NS = None


def _wrap_idx(flat):
    """Pack an int16 idx list into the SWDGE wrapped layout [128, n//16]:
    idx k lives at (partition k%16, col k//16), replicated across the eight
    16-partition groups."""
    n = flat.shape[0]
    assert n % 16 == 0
    w = flat.reshape(n // 16, 16).T.astype(np.int16)   # [16, n//16]
    return np.tile(w, (8, 1))


def _host_prep(x, edge_index):
    N = x.shape[0]
    E = edge_index.shape[1]
    Nl = N // NC
    GC = -(-Nl // P)
    Nlp = P * GC
    NT = NC * Nlp
    QS = NT // 4

    src = np.ascontiguousarray(edge_index[0]).astype(np.int64)
    dst = np.ascontiguousarray(edge_index[1]).astype(np.int64)
    deg = np.bincount(dst, minlength=N).astype(np.int64)
    xf = np.asarray(x[:, 0], np.float32)
    D1 = int(deg.max()) + 1          # self + in-neighbours, fixed width

    # degree-sorted local ranks
    n_all = np.arange(N)
    c_all = n_all // Nl
    rho = np.empty(N, np.int64)
    for c in range(NC):
        ids = np.arange(c * Nl, (c + 1) * Nl)
        order = np.argsort(-deg[ids], kind="stable")
        rho[ids[order]] = np.arange(Nl)
    gid = c_all * Nlp + rho
    w_all = rho // P
    p_all = rho % P

    # grids [NC, P, GC]
    x_grid = np.zeros((NC, P, GC), np.float32)
    x_grid[c_all, p_all, w_all] = xf
    deg_grid = np.zeros((NC, P, GC), np.int64)
    deg_grid[c_all, p_all, w_all] = deg

    # phase A/B slot geometry: Kb[w] = max degree in window (over cores)
    Kb = deg_grid.max(axis=1).max(axis=0)              # [GC]
    SKB = int(Kb.sum())
    SKBp = SKB
    baseB = np.zeros(GC + 1, np.int64)
    np.cumsum(Kb, out=baseB[1:])

    # per-edge rank within destination
    okey = np.argsort(gid[dst], kind="stable")
    sd = gid[dst][okey]
    erank_o = np.arange(E) - np.searchsorted(sd, sd)
    erank = np.empty(E, np.int64)
    erank[okey] = erank_o

    dcore = dst // Nl
    dw = rho[dst] // P
    dp = rho[dst] % P
    slot_col = baseB[dw] + erank                       # [E]

    # 2-hop neighbourhood table: row n = [x[n], x of in-neighbours, 0 pad]
    nbr_x = np.zeros((N, D1), np.float16)
    nbr_x[:, 0] = xf.astype(np.float16)
    nbr_x[dst, 1 + erank] = xf[src].astype(np.float16)

    # phase C tile geometry
    q_src = gid[src] // QS                              # [E] 0..3
    ewq = np.zeros((NC, GC, 4), np.int64)
    np.add.at(ewq, (dcore, dw, q_src), 1)
    Kwq = -(-ewq.max(axis=0) // P)                      # [GC, 4]

    # window groups (greedy by total tiles)
    wtiles = Kwq.sum(axis=1)                            # [GC]
    groups = []
    w0 = 0
    while w0 < GC:
        w1 = w0 + 1
        tot = int(wtiles[w0])
        while w1 < GC and tot + int(wtiles[w1]) <= TMAX:
            tot += int(wtiles[w1])
            w1 += 1
        groups.append((w0, w1))
        w0 = w1

    # tile column order: group -> quarter -> window
    col_of = np.zeros((GC, 4), np.int64)
    calls = []          # (grp_idx, q, colbase, ntiles) per dma_gather call
    grp_info = []       # per group: (w0, w1, colbase, ntiles)
    off = 0
    for gi, (w0, w1) in enumerate(groups):
        gbase = off
        for q in range(4):
            qbase = off
            for w in range(w0, w1):
                col_of[w, q] = off
                off += int(Kwq[w, q])
            ntq = off - qbase
            t0 = qbase
            while ntq > 0:
                n = min(ntq, CALL_TILES)
                calls.append((gi, q, t0, n))
                t0 += n
                ntq -= n
        grp_info.append((w0, w1, gbase, off - gbase))
    T = off

    # per-edge tile slots: rank within (core, window, quarter)
    okey2 = np.argsort((gid[dst] * 4 + q_src), kind="stable")
    sk2 = (gid[dst] * 4 + q_src)[okey2]
    # rank within (dst, quarter) -- but we need rank within (core, window,
    # quarter) across all dsts of the window.  Sort by (core, w, q, anything):
    key3 = (dcore * GC + dw) * 4 + q_src
    okey3 = np.argsort(key3, kind="stable")
    sk3 = key3[okey3]
    rank3_o = np.arange(E) - np.searchsorted(sk3, sk3)
    rank3 = np.empty(E, np.int64)
    rank3[okey3] = rank3_o
    tile_of = col_of[dw, q_src] + rank3 // P
    tslot = rank3 % P

    meta = []
    for c in range(NC):
        em = dcore == c
        sc, dc_ = src[em], dst[em]
        spc, col = dp[em], slot_col[em]

        # node-grid 2-hop arrays (phase A: per-node g)
        ids = np.arange(c * Nl, (c + 1) * Nl)
        x2hN = np.zeros((P, GC, D1), np.float16)
        dcntN = np.ones((P, GC), np.float16)
        x2hN[p_all[ids], w_all[ids]] = nbr_x[ids]
        dcntN[p_all[ids], w_all[ids]] = (deg[ids] + 1).astype(np.float16)

        # edge-slot 2-hop arrays (phase B: per-edge g[src])
        x2h = np.zeros((P, SKBp, D1), np.float16)
        dcnt = np.ones((P, SKBp), np.float16)
        x2h[spc, col] = nbr_x[sc]
        dcnt[spc, col] = (deg[sc] + 1).astype(np.float16)

        cti, csl = tile_of[em], tslot[em]
        cidx = np.zeros((P, T), np.int64)
        cdlo = np.full((P, T), 200.0, np.float16)
        cdinv = np.zeros((P, T), np.float16)
        cidx[csl, cti] = gid[sc] - q_src[em] * QS
        cdlo[csl, cti] = dp[em].astype(np.float16)
        cdinv[csl, cti] = (1.0 / np.maximum(deg[dc_], 1)).astype(np.float16)
        cidx_p = _wrap_idx(cidx.T.reshape(-1))          # [128, T*8]

        meta.append(dict(x2hN=x2hN.reshape(P, GC * D1),
                         dcntN=dcntN,
                         x2h=x2h.reshape(P, SKBp * D1),
                         dcnt=dcnt,
                         cidx=cidx_p, cdlo=cdlo, cdinv=cdinv,
                         deg_inv=np.where(deg_grid[c] > 0,
                                          1.0 / np.maximum(deg_grid[c], 1),
                                          1.0).astype(np.float32)))

    # per-window tile runs for the builder
    wruns = []
    for w in range(GC):
        runs = [(int(col_of[w, q]), int(Kwq[w, q]))
                for q in range(4) if Kwq[w, q] > 0]
        wruns.append(runs)

    # phase-B window chunks (bounded slot columns per chunk)
    CCOL = 128
    bchunks = []
    w0 = 0
    while w0 < GC:
        w1 = w0 + 1
        tot = int(Kb[w0])
        while w1 < GC and tot + int(Kb[w1]) <= CCOL:
            tot += int(Kb[w1])
            w1 += 1
        bchunks.append((w0, w1, int(baseB[w0]), tot))
        w0 = w1

    layout = dict(N=N, Nl=Nl, GC=GC, Nlp=Nlp, NT=NT, QS=QS, D1=D1,
                  SKB=SKB, SKBp=SKBp, Kb=Kb, baseB=baseB, bchunks=bchunks,
                  T=T, calls=calls, grp_info=grp_info, wruns=wruns,
                  gid=gid)
    return meta, layout


def _build_program(layout, H1, H2, OUT):
    GC, Nlp, NT, SKBp = layout["GC"], layout["Nlp"], layout["NT"], layout["SKBp"]
    SKB, Kb, baseB = layout["SKB"], layout["Kb"], layout["baseB"]
    T, calls, grp_info, wruns = (layout["T"], layout["calls"],
                                 layout["grp_info"], layout["wruns"])
    D1, bchunks = layout["D1"], layout["bchunks"]
    CCOL = max(cc for (_, _, _, cc) in bchunks)
    KH = H1 // P
    maxkw = max((sum(k for _, k in runs) for runs in wruns), default=1) or 1

    nc = bacc.Bacc("TRN2", target_bir_lowering=False, debug=False,
                   num_devices=NC, num_swdge_queues=4)

    def din(name, shape, dt):
        return nc.dram_tensor(name, shape, dt, kind="ExternalInput").ap()

    x2hN_t = din("x2hN", [P, GC * D1], F16)
    dcntN_t = din("dcntN", [P, GC], F16)
    x2h_t = din("x2h", [P, SKBp * D1], F16)
    dcnt_t = din("dcnt", [P, SKBp], F16)
    cidx_t = din("cidx", [P, T * 8], I16)
    cdlo_t = din("cdlo", [P, T], F16)
    cdinv_t = din("cdinv", [P, T], F16)
    deg_inv_t = din("deg_inv", [P, GC], F32)
    W1_t = din("W1", [1, H1], F32)
    att_s_t = din("att_src", [H1], F32)
    att_d_t = din("att_dst", [H1], F32)
    Wl1_t = din("Wl1", [H1, H2], F32)
    bl1_t = din("bl1", [H2], F32)
    Wr1_t = din("Wr1", [H1, H2], F32)
    Wl2_t = din("Wl2", [H2, OUT], F32)
    bl2_t = din("bl2", [OUT], F32)
    Wr2_t = din("Wr2", [H2, OUT], F32)
    out_t = nc.dram_tensor("out", [P, Nlp], F32, kind="ExternalOutput").ap()

    with tile.TileContext(nc) as tc:
        with (
            tc.tile_pool(name="dram", bufs=1, space="DRAM") as dram,
            tc.tile_pool(name="const", bufs=1) as constp,
            tc.tile_pool(name="grids", bufs=1) as gridp,
        ):
            # ---------------- phase 0: scalars and weight products ----------
            ph0 = tc.tile_pool(name="psum_s", bufs=2, space="PSUM")
            psum_s = ph0.__enter__()
            w_col = constp.tile([P, KH], F32)
            nc.sync.dma_start(w_col[:], W1_t.rearrange("o (j p) -> p (o j)", p=P))
            att_s = constp.tile([P, KH], F32)
            nc.sync.dma_start(att_s[:], att_s_t.rearrange("(j p) -> p j", p=P))
            att_d = constp.tile([P, KH], F32)
            nc.sync.dma_start(att_d[:], att_d_t.rearrange("(j p) -> p j", p=P))

            m23 = constp.tile([P, 2 * KH], F32)
            nc.vector.tensor_mul(out=m23[:, 0:KH], in0=w_col[:], in1=att_s[:])
            nc.vector.tensor_mul(out=m23[:, KH:2 * KH], in0=w_col[:], in1=att_d[:])
            ones_col = constp.tile([P, 1], F32)
            nc.vector.memset(ones_col[:], 1.0)
            csd_ps = psum_s.tile([1, 2 * KH], F32, space="PSUM")
            nc.tensor.matmul(csd_ps[:], lhsT=ones_col[:], rhs=m23[:], start=True, stop=True)
            csd4 = constp.tile([1, 2 * KH], F32)
            nc.vector.tensor_copy(out=csd4[:], in_=csd_ps[:])
            csd2 = constp.tile([1, 2], F32)
            nc.vector.tensor_reduce(
                out=csd2[:], in_=csd4[:].rearrange("o (a j) -> o a j", a=2),
                axis=mybir.AxisListType.X, op=Alu.add)
            ones_row = constp.tile([1, P], F32)
            nc.vector.memset(ones_row[:], 1.0)
            csd_bps = psum_s.tile([P, 2], F32, space="PSUM")
            nc.tensor.matmul(csd_bps[:], lhsT=ones_row[:], rhs=csd2[:], start=True, stop=True)
            csd_col = constp.tile([P, 2], F32)
            nc.vector.tensor_copy(out=csd_col[:], in_=csd_bps[:])
            cs_col = csd_col[:, 0:1]
            cd_col = csd_col[:, 1:2]
            cscd_col = constp.tile([P, 1], F32)
            nc.vector.tensor_add(out=cscd_col[:], in0=cs_col, in1=cd_col)

            # u/v columns and B5 = [u@Wl1; v@Wl1; u@Wr1; v@Wr1; bl1]
            uv = constp.tile([P, 2 * KH], F32)
            uvv = uv[:].rearrange("p (j two) -> p j two", two=2)
            nc.vector.tensor_scalar_max(out=uvv[:, :, 0], in0=w_col[:], scalar1=0.0)
            nc.vector.tensor_scalar(out=uvv[:, :, 1], in0=w_col[:], scalar1=-1.0,
                                    scalar2=0.0, op0=Alu.mult, op1=Alu.max)
            b5_dram = dram.tile([5, H2], F32)
            wlr = constp.tile([P, 2 * H2], F32, tag="wlr")
            abcd_ps = psum_s.tile([2, 2 * H2], F32, space="PSUM", tag="ab")
            for j in range(KH):
                nc.sync.dma_start(wlr[:, 0:H2], Wl1_t[j * P:(j + 1) * P, :])
                nc.sync.dma_start(wlr[:, H2:2 * H2], Wr1_t[j * P:(j + 1) * P, :])
                nc.tensor.matmul(abcd_ps[:], lhsT=uv[:, 2 * j:2 * j + 2], rhs=wlr[:],
                                 start=(j == 0), stop=(j == KH - 1))
            abcd_sb = constp.tile([2, 2 * H2], F32)
            nc.vector.tensor_copy(out=abcd_sb[:], in_=abcd_ps[:])
            nc.sync.dma_start(
                b5_dram[0:4, :].rearrange("(s r) f -> r s f", s=2),
                abcd_sb[:].rearrange("r (s f) -> r s f", s=2))
            nc.sync.dma_start(b5_dram[4:5, :], bl1_t.rearrange("(o f) -> o f", o=1))
            B5 = constp.tile([5, H2], F32)
            nc.sync.dma_start(B5[:], b5_dram[:])

            Wl2_h = constp.tile([H2, OUT], F16)
            wl2_f = constp.tile([H2, OUT], F32, tag="wtmp")
            nc.sync.dma_start(wl2_f[:], Wl2_t[:])
            nc.vector.tensor_copy(out=Wl2_h[:], in_=wl2_f[:])
            Wr2_h = constp.tile([H2, OUT], F16)
            wr2_f = constp.tile([H2, OUT], F32, tag="wtmp")
            nc.sync.dma_start(wr2_f[:], Wr2_t[:])
            nc.vector.tensor_copy(out=Wr2_h[:], in_=wr2_f[:])
            bl2_col = constp.tile([P, 1], F32)
            nc.sync.dma_start(bl2_col[:], bl2_t.rearrange("(p o) -> p o", o=1))

            iotaD_i = constp.tile([P, D1], mybir.dt.int32)
            nc.gpsimd.iota(iotaD_i[:], pattern=[[1, D1]], base=0, channel_multiplier=0)
            iotaD = constp.tile([P, D1], F16)
            nc.vector.tensor_copy(out=iotaD[:], in_=iotaD_i[:])
            iota128_i = constp.tile([P, P], mybir.dt.int32)
            nc.gpsimd.iota(iota128_i[:], pattern=[[1, P]], base=0, channel_multiplier=0)
            iota128h = constp.tile([P, P], F16)
            nc.vector.tensor_copy(out=iota128h[:], in_=iota128_i[:])
            identity = constp.tile([P, P], F32)
            from concourse.masks import make_identity
            make_identity(nc, identity[:])
            ph0.__exit__(None, None, None)

            # ---------------- persistent grids ----------------
            deg_inv = gridp.tile([P, GC], F32)
            nc.sync.dma_start(deg_inv[:], deg_inv_t[:])
            h2T = gridp.tile([P, Nlp], F16)

            h2_loc = dram.tile([Nlp, H2], F16)
            h2_tab = dram.tile([NC, Nlp, H2], F16, addr_space="Shared")
            c5_loc = dram.tile([5, Nlp], F32)

            # ---------------- phase A: per-node g via 2-hop slots ----------
            g_grid = gridp.tile([P, GC], F32)
            with tc.tile_pool(name="ph_a", bufs=1) as pa:
                x2n = pa.tile([P, GC * D1], F16)
                nc.sync.dma_start(x2n[:], x2hN_t[:])
                dcn = pa.tile([P, GC], F16)
                nc.sync.dma_start(dcn[:], dcntN_t[:])
                zN = pa.tile([P, GC * D1], F16)
                nc.vector.tensor_scalar(out=zN[:], in0=x2n[:], scalar1=cs_col,
                                        scalar2=None, op0=Alu.mult)
                x0v = (x2n[:].rearrange("p (c j) -> p c j", j=D1)[:, :, 0:1]
                       .to_broadcast([P, GC, D1]))
                zN3 = zN[:].rearrange("p (c j) -> p c j", j=D1)
                nc.vector.scalar_tensor_tensor(out=zN3, in0=x0v, scalar=cd_col,
                                               in1=zN3, op0=Alu.mult, op1=Alu.add)
                nc.vector.scalar_tensor_tensor(out=zN[:], in0=zN[:], scalar=NEG,
                                               in1=zN[:], op0=Alu.mult, op1=Alu.max)
                eeN = pa.tile([P, GC * D1], F32)
                nc.scalar.activation(eeN[:], zN[:], Act.Exp)
                mkN = pa.tile([P, GC * D1], F16)
                mkN3 = mkN[:].rearrange("p (c j) -> p c j", j=D1)
                nc.vector.tensor_tensor(
                    out=mkN3,
                    in0=iotaD[:].unsqueeze(1).to_broadcast([P, GC, D1]),
                    in1=dcn[:].unsqueeze(2).to_broadcast([P, GC, D1]),
                    op=Alu.is_lt)
                nc.vector.tensor_mul(out=eeN[:], in0=eeN[:], in1=mkN[:])
                SN = pa.tile([P, GC], F32)
                nc.vector.tensor_reduce(
                    out=SN[:], in_=eeN[:].rearrange("p (c j) -> p c j", j=D1),
                    axis=mybir.AxisListType.X, op=Alu.add)
                nc.vector.tensor_mul(out=eeN[:], in0=eeN[:], in1=x2n[:])
                WN = pa.tile([P, GC], F32)
                nc.vector.tensor_reduce(
                    out=WN[:], in_=eeN[:].rearrange("p (c j) -> p c j", j=D1),
                    axis=mybir.AxisListType.X, op=Alu.add)
                nc.vector.reciprocal(out=g_grid[:], in_=SN[:])
                nc.vector.tensor_mul(out=g_grid[:], in0=g_grid[:], in1=WN[:])

            # ---------------- phase B: per-edge g[src] via 2-hop slots ------
            Sp_grid = gridp.tile([P, GC], F32)
            Sq_grid = gridp.tile([P, GC], F32)
            with tc.tile_pool(name="ph_b", bufs=2) as pb, \
                 tc.tile_pool(name="ph_b1", bufs=1) as pb1, \
                 tc.tile_pool(name="psum_b", bufs=2, space="PSUM") as psum_b:
                pe_grid = pb1.tile([P, SKBp], F16)
                qe_grid = pb1.tile([P, SKBp], F16)
                for (w0b, w1b, c0, cc) in bchunks:
                    x2c = pb.tile([P, CCOL * D1], F16, tag="x2c")
                    nc.sync.dma_start(x2c[:, :cc * D1],
                                      x2h_t[:, c0 * D1:(c0 + cc) * D1])
                    dcc = pb.tile([P, CCOL], F16, tag="dcc")
                    nc.sync.dma_start(dcc[:, :cc], dcnt_t[:, c0:c0 + cc])
                    x2c3 = x2c[:, :cc * D1].rearrange("p (c j) -> p c j", j=D1)
                    zc = pb.tile([P, CCOL * D1], F16, tag="zc")
                    zc3 = zc[:, :cc * D1].rearrange("p (c j) -> p c j", j=D1)
                    nc.vector.tensor_scalar(out=zc[:, :cc * D1],
                                            in0=x2c[:, :cc * D1],
                                            scalar1=cs_col, scalar2=None,
                                            op0=Alu.mult)
                    x0c = x2c3[:, :, 0:1].to_broadcast([P, cc, D1])
                    nc.vector.scalar_tensor_tensor(out=zc3, in0=x0c,
                                                   scalar=cd_col, in1=zc3,
                                                   op0=Alu.mult, op1=Alu.add)
                    nc.vector.scalar_tensor_tensor(out=zc[:, :cc * D1],
                                                   in0=zc[:, :cc * D1],
                                                   scalar=NEG,
                                                   in1=zc[:, :cc * D1],
                                                   op0=Alu.mult, op1=Alu.max)
                    eec = pb.tile([P, CCOL * D1], F32, tag="eec")
                    nc.scalar.activation(eec[:, :cc * D1], zc[:, :cc * D1],
                                         Act.Exp)
                    zc3 = zc[:, :cc * D1].rearrange("p (c j) -> p c j", j=D1)
                    nc.vector.tensor_tensor(
                        out=zc3,
                        in0=iotaD[:].unsqueeze(1).to_broadcast([P, cc, D1]),
                        in1=dcc[:, :cc].unsqueeze(2).to_broadcast([P, cc, D1]),
                        op=Alu.is_lt)
                    nc.vector.tensor_mul(out=eec[:, :cc * D1],
                                         in0=eec[:, :cc * D1],
                                         in1=zc[:, :cc * D1])
                    Sc = pb.tile([P, CCOL], F32, tag="Sc")
                    eec3 = eec[:, :cc * D1].rearrange("p (c j) -> p c j", j=D1)
                    nc.vector.tensor_reduce(out=Sc[:, :cc], in_=eec3,
                                            axis=mybir.AxisListType.X,
                                            op=Alu.add)
                    nc.vector.tensor_mul(out=eec[:, :cc * D1],
                                         in0=eec[:, :cc * D1],
                                         in1=x2c[:, :cc * D1])
                    Wc = pb.tile([P, CCOL], F32, tag="Wc")
                    nc.vector.tensor_reduce(out=Wc[:, :cc], in_=eec3,
                                            axis=mybir.AxisListType.X,
                                            op=Alu.add)
                    gec = pb.tile([P, CCOL], F32, tag="gec")
                    nc.vector.reciprocal(out=gec[:, :cc], in_=Sc[:, :cc])
                    nc.vector.tensor_mul(out=gec[:, :cc], in0=gec[:, :cc],
                                         in1=Wc[:, :cc])
                    nc.vector.tensor_scalar_max(out=pe_grid[:, c0:c0 + cc],
                                                in0=gec[:, :cc], scalar1=0.0)
                    nc.vector.tensor_scalar(out=qe_grid[:, c0:c0 + cc],
                                            in0=gec[:, :cc], scalar1=-1.0,
                                            scalar2=0.0, op0=Alu.mult,
                                            op1=Alu.max)
                for w in range(GC):
                    kb = int(Kb[w])
                    o0 = int(baseB[w])
                    if kb == 0:
                        nc.vector.memset(Sp_grid[:, w:w + 1], 0.0)
                        nc.vector.memset(Sq_grid[:, w:w + 1], 0.0)
                        continue
                    nc.vector.tensor_reduce(
                        out=Sp_grid[:, w:w + 1],
                        in_=pe_grid[:, o0:o0 + kb].rearrange("p (o k) -> p o k", o=1),
                        axis=mybir.AxisListType.X, op=Alu.add)
                    nc.vector.tensor_reduce(
                        out=Sq_grid[:, w:w + 1],
                        in_=qe_grid[:, o0:o0 + kb].rearrange("p (o k) -> p o k", o=1),
                        axis=mybir.AxisListType.X, op=Alu.add)

                # coefficient grids -> transposed -> c5_loc rows
                cP = pb1.tile([P, GC], F32)
                nc.vector.tensor_mul(out=cP[:], in0=Sp_grid[:], in1=deg_inv[:])
                cQ = pb1.tile([P, GC], F32)
                nc.vector.tensor_mul(out=cQ[:], in0=Sq_grid[:], in1=deg_inv[:])
                cp = pb1.tile([P, GC], F32)
                nc.vector.tensor_scalar_max(out=cp[:], in0=g_grid[:], scalar1=0.0)
                cq = pb1.tile([P, GC], F32)
                nc.vector.tensor_scalar(out=cq[:], in0=g_grid[:], scalar1=-1.0,
                                        scalar2=0.0, op0=Alu.mult, op1=Alu.max)
                for j, grid in enumerate((cP, cQ, cp, cq)):
                    tp = psum_b.tile([GC, P], F32, space="PSUM", tag="tp")
                    nc.tensor.matmul(tp[:], lhsT=grid[:], rhs=identity[:],
                                     start=True, stop=True)
                    tps = pb.tile([GC, P], F32, tag="tps")
                    nc.vector.tensor_copy(out=tps[:], in_=tp[:])
                    nc.sync.dma_start(
                        c5_loc[j:j + 1, :].rearrange("o (w e) -> (o w) e", e=P),
                        tps[:])
                ones_t = pb1.tile([GC, P], F32)
                nc.vector.memset(ones_t[:], 1.0)
                nc.sync.dma_start(
                    c5_loc[4:5, :].rearrange("o (w e) -> (o w) e", e=P),
                    ones_t[:])

            # ---------------- h2 build (local nodes only) ----------------
            with tc.tile_pool(name="h2p", bufs=3) as h2p, \
                 tc.tile_pool(name="h2c", bufs=1) as h2c, \
                 tc.tile_pool(name="psum_h", bufs=3, space="PSUM") as psum_h:
                c5_sb = h2c.tile([5, Nlp], F32)
                nc.sync.dma_start(c5_sb[:], c5_loc[:])
                GB = 4
                for w0 in range(0, GC, GB):
                    nw = min(GB, GC - w0)
                    hp = psum_h.tile([P, GB * H2], F32, space="PSUM", tag="hp")
                    for j in range(nw):
                        w = w0 + j
                        nc.tensor.matmul(
                            hp[:, j * H2:(j + 1) * H2],
                            lhsT=c5_sb[:, w * P:(w + 1) * P],
                            rhs=B5[:], start=True, stop=True)
                    ht = h2p.tile([P, GB * H2], F16, tag="ht")
                    nc.vector.tensor_scalar_max(out=ht[:, :nw * H2],
                                                in0=hp[:, :nw * H2], scalar1=0.0)
                    nc.sync.dma_start(
                        h2_loc[w0 * P:(w0 + nw) * P, :]
                            .rearrange("(j p) f -> p j f", p=P),
                        ht[:, :nw * H2].rearrange("p (j f) -> p j f", f=H2))
                # transposed local h2 for the Wr2 term
                for a in range(0, Nlp, 512):
                    wd = min(512, Nlp - a)
                    hp2 = psum_h.tile([P, 512], F32, space="PSUM", tag="hp2")
                    nc.tensor.matmul(hp2[:, :wd], lhsT=B5[:], rhs=c5_sb[:, a:a + wd],
                                     start=True, stop=True)
                    nc.vector.tensor_scalar_max(out=h2T[:, a:a + wd],
                                                in0=hp2[:, :wd], scalar1=0.0)

            nc.gpsimd.collective_compute(
                "AllGather", Alu.bypass,
                replica_groups=[list(range(NC))],
                ins=[h2_loc.opt()], outs=[h2_tab.opt()])

            # ---------------- phase C ----------------
            with tc.tile_pool(name="ph_c", bufs=2) as pc, \
                 tc.tile_pool(name="ph_cm", bufs=3) as pcm, \
                 tc.tile_pool(name="ph_c1", bufs=1) as pc1, \
                 tc.tile_pool(name="psum_c", bufs=4, space="PSUM") as psum_c:
                cdlo_sb = pc1.tile([P, T], F16)
                nc.sync.dma_start(cdlo_sb[:], cdlo_t[:])
                cdinv_sb = pc1.tile([P, T], F16)
                nc.sync.dma_start(cdinv_sb[:], cdinv_t[:])

                h2q = [h2_tab[2 * q:2 * q + 2].rearrange("a r e -> (a r) e")
                       for q in range(4)]
                calls_by_grp = {}
                for (gi, q, t0, ntl) in calls:
                    calls_by_grp.setdefault(gi, []).append((q, t0, ntl))

                _gq = [0]
                for gi, (w0, w1, gbase, gtiles) in enumerate(grp_info):
                    if gtiles == 0:
                        vt = None
                    else:
                        vt = pc.tile([P, TMAX * P], F16, tag="vt")
                        for (q, t0, ntl) in calls_by_grp.get(gi, []):
                            ci = pcm.tile([P, CALL_TILES * 8], I16, tag="ci")
                            nc.sync.dma_start(ci[:, :ntl * 8],
                                              cidx_t[:, t0 * 8:(t0 + ntl) * 8])
                            nc.gpsimd.dma_gather(
                                vt[:, (t0 - gbase) * P:(t0 - gbase + ntl) * P]
                                    .rearrange("p (t e) -> p t e", e=P),
                                h2q[q], ci[:, :ntl * 8],
                                num_idxs=ntl * P, num_idxs_reg=ntl * P,
                                elem_size=P, single_packet=False,
                                queue_num=_gq[0] % 4)
                            _gq[0] += 1
                        # scale by 1/deg (zeroes padding slots)
                        nc.vector.tensor_tensor(
                            out=vt[:, :gtiles * P].rearrange("p (t e) -> p t e", e=P),
                            in0=vt[:, :gtiles * P].rearrange("p (t e) -> p t e", e=P),
                            in1=cdinv_sb[:, gbase:gbase + gtiles].unsqueeze(2)
                                .to_broadcast([P, gtiles, P]),
                            op=Alu.mult)
                    for w in range(w0, w1):
                        runs = wruns[w]
                        ktot = sum(k for _, k in runs)
                        if ktot > 0:
                            mt = pcm.tile([P, maxkw * P], F16, tag="mt")
                            mo = 0
                            for (t0, k) in runs:
                                nc.vector.tensor_tensor(
                                    out=mt[:, mo * P:(mo + k) * P]
                                        .rearrange("p (t e) -> p t e", e=P),
                                    in0=cdlo_sb[:, t0:t0 + k].unsqueeze(2)
                                        .to_broadcast([P, k, P]),
                                    in1=iota128h[:].unsqueeze(1)
                                        .to_broadcast([P, k, P]),
                                    op=Alu.is_equal)
                                mo += k
                            yp = psum_c.tile([P, P], F32, space="PSUM", tag="yp")
                            mo = 0
                            ti = 0
                            for (t0, k) in runs:
                                for t in range(k):
                                    nc.tensor.matmul(
                                        yp[:],
                                        lhsT=vt[:, (t0 - gbase + t) * P:
                                                (t0 - gbase + t + 1) * P],
                                        rhs=mt[:, (mo + t) * P:(mo + t + 1) * P],
                                        start=(ti == 0),
                                        stop=(ti == ktot - 1))
                                    ti += 1
                                mo += k
                            ys = pcm.tile([P, P], F16, tag="ys")
                            nc.vector.tensor_copy(out=ys[:], in_=yp[:])
                        op = psum_c.tile([P, P], F32, space="PSUM", tag="op")
                        if ktot > 0:
                            nc.tensor.matmul(op[:], lhsT=Wl2_h[:], rhs=ys[:],
                                             start=True, stop=False)
                            nc.tensor.matmul(op[:], lhsT=Wr2_h[:],
                                             rhs=h2T[:, w * P:(w + 1) * P],
                                             start=False, stop=True)
                        else:
                            nc.tensor.matmul(op[:], lhsT=Wr2_h[:],
                                             rhs=h2T[:, w * P:(w + 1) * P],
                                             start=True, stop=True)
                        ow = pcm.tile([P, P], F32, tag="ow")
                        nc.scalar.activation(ow[:], op[:], Act.Identity,
                                             bias=bl2_col[:])
                        nc.sync.dma_start(out_t[:, w * P:(w + 1) * P], ow[:])

    nc.compile()
    return nc


def kernel(**inputs):
    x = np.asarray(inputs["x"], np.float32)
    edge_index = np.asarray(inputs["edge_index"])
    b1 = np.asarray(inputs["b1"], np.float32)
    assert float(np.abs(b1).max()) == 0.0, "kernel factorization requires b1 == 0"

    meta, layout = _host_prep(x, edge_index)
    H1 = inputs["W1"].shape[1]
    H2 = inputs["Wl1"].shape[1]
    OUT = inputs["Wl2"].shape[1]

    nc = _build_program(layout, H1, H2, OUT)

    shared = dict(
        W1=np.asarray(inputs["W1"], np.float32),
        att_src=np.asarray(inputs["att_src"], np.float32),
        att_dst=np.asarray(inputs["att_dst"], np.float32),
        Wl1=np.asarray(inputs["Wl1"], np.float32),
        bl1=np.asarray(inputs["bl1"], np.float32),
        Wr1=np.asarray(inputs["Wr1"], np.float32),
        Wl2=np.asarray(inputs["Wl2"], np.float32),
        bl2=np.asarray(inputs["bl2"], np.float32),
        Wr2=np.asarray(inputs["Wr2"], np.float32),
    )
    in_maps = []
    for c in range(NC):
        m = dict(shared)
        for k2 in ("x2hN", "dcntN", "x2h", "dcnt", "cidx", "cdlo",
                   "cdinv", "deg_inv"):
            m[k2] = meta[c][k2]
        in_maps.append(m)

    trace = bool(os.environ.get("KERNEL_TRACE"))
    if trace:
        try:
            import trn_agent_boot.trn_boot as _tb
            try:
                from antenv.axon_hooks import set_axon_ntff_profile_hook
            except ImportError:
                import types
                import antenv
                _m = types.ModuleType("antenv.axon_hooks")
                _h = {}
                _m.set_axon_ntff_profile_hook = lambda hk: _h.__setitem__("h", hk)
                _m.get_axon_ntff_profile_hook = lambda: _h.get("h")
                sys.modules["antenv.axon_hooks"] = _m
                antenv.axon_hooks = _m
                set_axon_ntff_profile_hook = _m.set_axon_ntff_profile_hook

            set_axon_ntff_profile_hook(
                _tb._ntff_profile_via_ctypes("/opt/axon/libaxon_pjrt.so"))
        except Exception:
            trace = False
    res = run_bass_kernel_spmd(nc, in_maps, core_ids=list(range(NC)), trace=trace)
    global LAST_EXEC_NS
    LAST_EXEC_NS = res.exec_time_ns

    N, Nlp, gid = layout["N"], layout["Nlp"], layout["gid"]
    full = np.concatenate([res.results[c]["out"].T for c in range(NC)], axis=0)
    return np.ascontiguousarray(full[gid]).astype(np.float32)
